# revision 1
# baseline (speedup 1.0000x reference)
"""Depth-weighted 3x3 conv (DepthConv) Trainium2 Bass kernel.

out[b,o,h,w] = sum_{c,i,j} img_pad[b,c,h+i,w+j] * exp(-8.3*|d[b,h,w]-d_pad[b,h+i,w+j]|)
               * weight[o,c,i,j]  + bias[o]

Sharding: data-parallel over batch, one batch element per NeuronCore (8 cores).

Per-core plan (all heavy compute on device):
  - The 3x3x64 = 576-row contraction is split into 5 K-chunks:
    4 "pair" chunks of 128 rows = 2 taps x 64 channels, plus the center tap
    (dw == 1) as a 64-row chunk read straight from the padded bf16 image.
  - Pair chunks' rhs (z = img_patch * dw) is built by DVE tensor-tensor
    multiplies at bf16 2x mode.  The per-pixel depth weight dw is staged in
    DRAM scratch and broadcast across the 64 channel partitions by merged
    HBM-read DMAs (0-stride repeat dims), split across both HWDGE rings, so
    the replica re-reads ride HBM bandwidth instead of the SBUF AXI ports.
  - Taps are paired so that the two halves of a 128-partition TT op can use
    ONE uniform free-dim offset: the bottom 64 partitions hold a copy of the
    padded image placed at a byte offset shifted by (delta_top - delta_bot).
    Pairs: (t0,t1) and (t7,t8) share shift 1 (tile imgA),
           (t2,t3) and (t5,t6) share shift 128 (tile imgB).
  - Matmuls accumulate the 5 chunks into PSUM [64 out-ch, 512 px] (fp32),
    bias is added by the scalar engine (per-partition bias), result DMAd out.
"""

import numpy as np


def _setup_path():
    try:
        import concourse.bass  # noqa: F401
    except ImportError:
        import sys

        for p in ("/opt/trn_rl_repo", "/root/.axon_site/_ro/trn_rl_repo"):
            if p not in sys.path:
                sys.path.insert(0, p)


_setup_path()

import ml_dtypes  # noqa: E402
import concourse.bass as bass  # noqa: E402
import concourse.mybir as mybir  # noqa: E402
import concourse.tile as tile  # noqa: E402
from concourse.bass_utils import run_bass_kernel_spmd  # noqa: E402
from concourse.mybir import (  # noqa: E402
    ActivationFunctionType as ACTF,
    AluOpType as ALU,
)

dt = mybir.dt

B, CIN, COUT, H, W = 8, 64, 64, 128, 128
HW = H * W  # 16384
WP = 130  # padded row length
PADSZ = WP * WP  # 16900
ALPHA = 8.3
N_CORES = 8

# tap t = 3*i + j ; padded-flat offset delta_t = 130*i + j
DELTA = [130 * i + j for i in range(3) for j in range(3)]
# pairs (top_tap, bottom_tap): top read at AP, bottom at AP - (d_top - d_bot)
#   imgA (bottom copy shifted by 1):   (0,1)  base 1+d0   ; (7,8)  base 1+d7
#   imgB (bottom copy shifted by 128): (2,3)  base 128+d2 ; (5,6)  base 128+d5
PAIRS = [(0, 1), (7, 8), (2, 3), (5, 6)]  # -> chunks 0..3 ; chunk 4 = center
PAIR_IMG = ["A", "A", "B", "B"]
PAIR_BASE = [1 + DELTA[0], 1 + DELTA[7], 128 + DELTA[2], 128 + DELTA[5]]
CENTER_BASE = 1 + DELTA[4]  # read from imgA top half
# dw slot s in the [128, 8*128] dw tile, in pair order (top, bottom):
SLOT_TAPS = [0, 1, 7, 8, 2, 3, 5, 6]

GROUPS = 8  # dw-broadcast groups
GPX = HW // GROUPS  # 2048 px per group
TILES = 4  # matmul tiles per group
TPX = GPX // TILES  # 512 px per tile (= 4 image rows)

IMGA_LEN = 1 + PADSZ + 8  # top copy at offset 1, bottom at 0 (+slack)
IMGB_LEN = 128 + PADSZ + 8  # top copy at offset 128, bottom at 0


def _bcast(row_ap, nrep):
    """[1, N] single-partition AP -> [1, nrep, N] with 0-stride repeat dim.

    Legal as a DMA source (free-dim 0-stride); used to fan one dw row out
    across `nrep` destination partitions.
    """
    from concourse.bass_types import AP

    p0 = list(row_ap.ap[0])
    fr = list(row_ap.ap[-1])
    return AP(tensor=row_ap.tensor, offset=row_ap.offset, ap=[p0, [0, nrep], fr])


def _win(ap_tile, prange, base, nrows=4):
    """[P, nrows, 128] window AP over a padded flat image tile."""
    v = ap_tile[prange[0] : prange[1], base : base + nrows * WP]
    return v.rearrange("p (r w) -> p r w", r=nrows)[:, :, 0:W]


def _body(tc, img_d, dep_d, wt_d, bias_d, out_d, reps=1):
    nc = tc.nc
    f32, bf16 = dt.float32, dt.bfloat16

    out_flat = out_d.rearrange("o h w -> o (h w)")

    with (
        tc.tile_pool(name="big", bufs=1) as big,
        tc.tile_pool(name="consts", bufs=1) as consts,
        tc.tile_pool(name="dw", bufs=1) as dwp,
        tc.tile_pool(name="dwb", bufs=2) as dwbp,
        tc.tile_pool(name="z", bufs=3) as zp,
        tc.tile_pool(name="osb", bufs=3) as osb,
        tc.tile_pool(name="psum", bufs=4, space="PSUM") as psp,
    ):
        # ---- constants -------------------------------------------------
        wc = consts.tile([128, 5 * 64], bf16, tag="wc")
        for k in range(5):
            nc.sync.dma_start(wc[:, k * 64 : (k + 1) * 64], wt_d[k])
        bias_t = consts.tile([64, 1], f32, tag="bias")
        nc.sync.dma_start(bias_t[:, :], bias_d)

        # ---- depth: Dall[h, i*130+jw] = depth_pad[h+i, jw], one DMA ----
        Dall = consts.tile([128, 3 * WP], f32, tag="Dall")
        from concourse.bass_types import AP as _AP

        dep_src = _AP(
            tensor=dep_d.tensor,
            offset=0,
            ap=[[WP, 128], [WP, 3], [1, WP]],
        )
        nc.sync.dma_start(
            Dall[:, :].rearrange("p (i jw) -> p i jw", i=3), dep_src
        )

        # ---- dw = exp(-8.3*|dp - center|), layout [h, slot*128+w] ------
        dwd = dwp.tile([128, 8 * W], f32, tag="dwd")
        center = Dall[:, WP + 1 : WP + 1 + W]
        for s, t in enumerate(SLOT_TAPS):
            i, j = divmod(t, 3)
            nc.vector.tensor_sub(
                dwd[:, s * W : (s + 1) * W], Dall[:, i * WP + j : i * WP + j + W], center
            )
        dw9 = dwp.tile([128, 8 * W], bf16, tag="dw9")
        HW4 = 4 * W
        for h in range(2):
            sl = slice(h * HW4, (h + 1) * HW4)
            nc.scalar.activation(dwd[:, sl], dwd[:, sl], ACTF.Abs, scale=-ALPHA)
            nc.scalar.activation(dw9[:, sl], dwd[:, sl], ACTF.Exp, scale=-1.0)

        # scramble to px-major, straight to DRAM scratch:
        # dwT_hbm[s, px] = dw of tap-slot s at pixel px
        dwT_d = nc.dram_tensor("dwT_scratch", (8, HW), dt.bfloat16, kind="Internal").ap()
        for s in range(8):
            nc.sync.dma_start(
                dwT_d[s : s + 1, :].rearrange("s (h w) -> s h w", h=H),
                dw9[:, s * W : (s + 1) * W],
            )

        # ---- padded bf16 image copies ----------------------------------
        # imgA: top(0:64)@1, bottom(64:128)@0 ; imgB: top@128, bottom@0
        imgA = big.tile([128, IMGA_LEN], bf16, tag="imgA")
        imgB = big.tile([128, IMGB_LEN], bf16, tag="imgB")

        # imgA-top: HBM load with fp32 -> bf16 cast during DMA (SWDGE);
        # the image arrives pre-padded from the host.
        # chunked loads/copies: sub-tile deps let early groups start while
        # the lower image half is still in flight
        HALF = 65 * WP
        nc.gpsimd.dma_start(imgA[0:64, 1 : 1 + HALF], img_d[:, 0:HALF])
        nc.gpsimd.dma_start(imgA[0:64, 1 + HALF : 1 + PADSZ], img_d[:, HALF:PADSZ])
        nc.vector.tensor_copy(
            imgB[0:64, 128 : 128 + HALF], imgA[0:64, 1 : 1 + HALF]
        )
        nc.vector.tensor_copy(
            imgB[0:64, 128 + HALF : 128 + PADSZ], imgA[0:64, 1 + HALF : 1 + PADSZ]
        )
        for lo, hi in ((0, HALF), (HALF, PADSZ)):
            nc.sync.dma_start(imgA[64:128, lo:hi], imgA[0:64, 1 + lo : 1 + hi])
            nc.sync.dma_start(imgB[64:128, lo:hi], imgA[0:64, 1 + lo : 1 + hi])

        # ---- main loop --------------------------------------------------
        img_tiles = {"A": imgA, "B": imgB}
        for g in range(GROUPS * reps):
            g = g % GROUPS
            dwb = [dwbp.tile([128, GPX], bf16, tag=f"dwb{p}", name=f"dwb{p}") for p in range(4)]
            for p in range(4):
                # one DMA per pair-block: HBM-read broadcast of slots 2p, 2p+1
                # src dims (half: 2 taps, rep: 64, px); dst [128, GPX]
                dsrc = _AP(
                    tensor=dwT_d.tensor,
                    offset=(2 * p) * HW + g * GPX,
                    ap=[[HW, 2], [0, 64], [1, GPX]],
                )
                (nc.sync if p % 2 == 0 else nc.scalar).dma_start(dwb[p][:, :], dsrc)
            for it in range(TILES):
                px0 = g * GPX + it * TPX
                r0 = px0 // W  # first image row of this tile
                zs = []
                for p in range(4):
                    z = zp.tile([128, TPX], bf16, tag=f"z{p}", name=f"z{p}")
                    nc.vector.tensor_mul(
                        z[:, :].rearrange("p (r w) -> p r w", w=W),
                        _win(img_tiles[PAIR_IMG[p]], (0, 128), PAIR_BASE[p] + r0 * WP),
                        dwb[p][:, it * TPX : (it + 1) * TPX].rearrange(
                            "p (r w) -> p r w", w=W
                        ),
                    )
                    zs.append(z)
                ps = psp.tile([64, TPX], f32, tag="ps")
                nc.tensor.matmul(
                    ps[:, :],
                    wc[0:64, 4 * 64 : 5 * 64],
                    _win(imgA, (0, 64), CENTER_BASE + r0 * WP),
                    start=True,
                    stop=False,
                )
                for p in range(4):
                    nc.tensor.matmul(
                        ps[:, :],
                        wc[:, p * 64 : (p + 1) * 64],
                        zs[p][:, :].rearrange("p (r w) -> p r w", w=W),
                        start=False,
                        stop=(p == 3),
                    )
                ob = osb.tile([64, TPX], f32, tag="ob")
                nc.scalar.activation(ob[:, :], ps[:, :], ACTF.Identity, bias=bias_t[:, 0:1])
                nc.scalar.dma_start(out_flat[:, px0 : px0 + TPX], ob[:, :])


def _split_multiwaits(nc):
    """TRN2 codegen allows a single sync-wait per instruction; Tile can emit
    more at multi-producer joins.  Move surplus waits onto standalone
    EventSemaphore instructions just before the instruction, same engine."""
    n = 0
    for fn in nc.m.functions:
        for blk in fn.blocks:
            idx = 0
            while idx < len(blk.instructions):
                inst = blk.instructions[idx]
                si = inst.sync_info
                if si is not None and len(si.on_wait) > 1:
                    waits = list(si.on_wait)
                    for w in waits[:-1]:
                        ev = mybir.InstEventSemaphore(
                            name=f"wsplit-{nc.next_id()}",
                            ins=[],
                            outs=[],
                            sync_info=mybir.SyncInfo(on_wait=[w], on_update=[]),
                        )
                        ev.engine = inst.engine
                        nc.register_instruction(ev)
                        blk.instructions.insert(idx, ev)
                        idx += 1
                        n += 1
                    inst.sync_info = mybir.SyncInfo(
                        on_wait=[waits[-1]], on_update=list(si.on_update)
                    )
                idx += 1
    return n


_CACHE = {}


def _build(reps=1):
    key = ("nc", reps)
    if key not in _CACHE:
        nc = bass.Bass(
            "TRN2", target_bir_lowering=False, debug=False, num_devices=N_CORES
        )
        img_d = nc.dram_tensor("img", (CIN, PADSZ), dt.float32, kind="ExternalInput").ap()
        dep_d = nc.dram_tensor("depth", (WP, WP), dt.float32, kind="ExternalInput").ap()
        wt_d = nc.dram_tensor("wt", (5, 128, 64), dt.bfloat16, kind="ExternalInput").ap()
        bias_d = nc.dram_tensor("bias", (64, 1), dt.float32, kind="ExternalInput").ap()
        out_d = nc.dram_tensor("out", (COUT, H, W), dt.float32, kind="ExternalOutput").ap()
        with tile.TileContext(nc) as tc:
            _body(tc, img_d, dep_d, wt_d, bias_d, out_d, reps=reps)
        _split_multiwaits(nc)
        _CACHE[key] = nc
    return _CACHE[key]


def _host_weights(weight):
    w = np.asarray(weight, dtype=np.float32)  # [o, c, i, j]
    wt = np.zeros((5, 128, 64), dtype=np.float32)
    for k, (ta, tb) in enumerate(PAIRS):
        wt[k, 0:64, :] = w[:, :, ta // 3, ta % 3].T
        wt[k, 64:128, :] = w[:, :, tb // 3, tb % 3].T
    wt[4, 0:64, :] = w[:, :, 1, 1].T
    return wt.astype(ml_dtypes.bfloat16)


def make_in_maps(img, depth, weight, bias):
    wt = _host_weights(weight)
    bias_h = np.asarray(bias, dtype=np.float32).reshape(COUT, 1)
    img = np.asarray(img, dtype=np.float32)
    depth = np.asarray(depth, dtype=np.float32)
    img_p = np.pad(img, ((0, 0), (0, 0), (1, 1), (1, 1))).reshape(B, CIN, PADSZ)
    dep_p = np.pad(depth[:, 0], ((0, 0), (1, 1), (1, 1)))
    return [
        {
            "img": np.ascontiguousarray(img_p[b]),
            "depth": np.ascontiguousarray(dep_p[b]),
            "wt": wt,
            "bias": bias_h,
        }
        for b in range(B)
    ]


def _runner(reps=1):
    """Persistent jitted 8-core executor (compile once per process)."""
    rkey = ("run", reps)
    if rkey in _CACHE:
        return _CACHE[rkey]
    import jax
    from jax.sharding import Mesh, PartitionSpec
    from jax.experimental.shard_map import shard_map
    from concourse.bass2jax import (
        _bass_exec_p,
        install_neuronx_cc_hook,
        partition_id_tensor,
    )

    nc = _build(reps=reps)
    install_neuronx_cc_hook()

    pid_name = nc.partition_id_tensor.name if nc.partition_id_tensor else None
    in_names, out_names, out_avals = [], [], []
    for alloc in nc.m.functions[0].allocations:
        if not isinstance(alloc, mybir.MemoryLocationSet):
            continue
        name = alloc.memorylocations[0].name
        if alloc.kind == "ExternalInput":
            if name != pid_name:
                in_names.append(name)
        elif alloc.kind == "ExternalOutput":
            out_names.append(name)
            out_avals.append(
                jax.core.ShapedArray(
                    tuple(alloc.tensor_shape), mybir.dt.np(alloc.dtype)
                )
            )
    n_params = len(in_names)
    all_in = in_names + out_names  # zero-init output buffers ride as inputs
    if pid_name is not None:
        all_in = all_in + [pid_name]
    donate = tuple(range(n_params, n_params + len(out_names)))

    def _bass_body(*args):
        operands = list(args)
        if pid_name is not None:
            operands.append(partition_id_tensor())
        return tuple(
            _bass_exec_p.bind(
                *operands,
                out_avals=tuple(out_avals),
                in_names=tuple(all_in),
                out_names=tuple(out_names),
                lowering_input_output_aliases=(),
                sim_require_finite=True,
                sim_require_nnan=True,
                nc=nc,
            )
        )

    devices = jax.devices()[:N_CORES]
    mesh = Mesh(np.asarray(devices), ("core",))
    nin = n_params + len(out_names)
    sharded = jax.jit(
        shard_map(
            _bass_body,
            mesh=mesh,
            in_specs=(PartitionSpec("core"),) * nin,
            out_specs=(PartitionSpec("core"),) * len(out_names),
            check_rep=False,
        ),
        donate_argnums=donate,
        keep_unused=True,
    )
    run = (sharded, in_names, out_names, out_avals, mesh)
    _CACHE[rkey] = run
    return run


def _concat_inputs(in_maps, in_names):
    return [
        np.concatenate([np.asarray(m[name]) for m in in_maps], axis=0)
        for name in in_names
    ]


def _zero_outs(out_avals):
    return [
        np.zeros((N_CORES * a.shape[0], *a.shape[1:]), a.dtype) for a in out_avals
    ]


def kernel(img, depth, weight, bias):
    sharded, in_names, out_names, out_avals, _ = _runner()
    in_maps = make_in_maps(img, depth, weight, bias)
    concat_in = _concat_inputs(in_maps, in_names)
    out_arrs = sharded(*concat_in, *_zero_outs(out_avals))
    oi = out_names.index("out")
    out = np.asarray(out_arrs[oi]).reshape(N_CORES, COUT, H, W)
    return out.astype(np.float32)

